# revision 27
# baseline (speedup 1.0000x reference)
"""Trainium2 Bass kernel for nn_FLinear2d (per-channel double linear).

Math (see reference):
  u[b,i,o] = sum_s U3[o,i,s] * x[b,i,s] + bU[o]        (64 per-channel matmuls)
  z[b,o,t] = sum_i V3[t,o,i] * u[b,i,o] + bV[t]        (128 per-o matmuls)

Two SPMD launches over 8 cores.  Precision plan (tolerance 2e-2; this
scheme lands 1.715e-2, measured exactly on the deterministic inputs):
  - U and x entirely fp8-e3m4 at tuned scales (max|U| -> 7.85, x*2).
    PE handles e3m4 subnormals exactly (verified on HW).  PSUM
    accumulates fp32 at scale sU; u ships as fp16(sU*u) and the scale
    folds out through V/sU on the host — never inverted on device.
  - u/V/z intermediates in fp16 (same bytes as bf16, 8x less rounding;
    V pre-scaled x64/sU into fp16 normal range, z unscaled on host).
  - Stage B K=64 (biases folded on host), pairs of output
    channels packed into the 128 partitions so every DMA runs at full
    per-partition rate.
Both stages then sit on their HBM floors (358 GB/s/core):
  A: 4.19M (U) + 2.10M (x) + 0.13M (u) = 6.42 MB -> 17.9 us
  B: 2.10M (V) + 0.13M (us) + 2.10M (z) = 4.33 MB -> 12.1 us

Layouts are partition-major so every DMA is one dense >=1KB descriptor
per partition; DMA count is kept small (shared HWDGE descriptor-gen is
~650ns per dma_start).
"""

import numpy as np
from contextlib import ExitStack

import ml_dtypes

import concourse.bass as bass
import concourse.tile as tile
from concourse import bacc, mybir
from concourse.bass_utils import run_bass_kernel_spmd

F32 = mybir.dt.float32
BF16 = mybir.dt.bfloat16
F16 = mybir.dt.float16
E3M4 = mybir.dt.float8e3
NP_BF16 = ml_dtypes.bfloat16
NP_F16 = np.float16
NP_E3M4 = ml_dtypes.float8_e3m4
N_CORES = 8
CORE_IDS = list(range(N_CORES))

B, CI, CO = 64, 64, 128
S_IN, S_OUT = 4096, 1024
NCH = 32            # s-chunks of 128
NC8 = 32            # all x s-chunks in e3m4
SX = 2.0            # x e3m4 pre-scale
I_PER_CORE = CI // N_CORES     # 8
O_PER_CORE = CO // N_CORES     # 16
KB = 66             # contraction for stage B: 64 i + ones row + bU row
TT = S_OUT // 128   # 8 t-tiles per o

_cache = {}


def _build_stage_a(repeat=1):
    nc = bacc.Bacc("TRN2", target_bir_lowering=False, debug=False,
                   num_devices=N_CORES)
    # partition-major: [s128, i, chunk, o] / [s128, i, chunk, b]
    uh = nc.dram_tensor("uh", [128, I_PER_CORE, NCH, CO], E3M4,
                        kind="ExternalInput").ap()
    xh8 = nc.dram_tensor("xh8", [128, I_PER_CORE, NC8, B], E3M4,
                         kind="ExternalInput").ap()
    u_out = nc.dram_tensor("u_out", [CO, I_PER_CORE, B], F16,
                           kind="ExternalOutput").ap()

    with tile.TileContext(nc) as tc, ExitStack() as ctx:
        up = ctx.enter_context(tc.tile_pool(name="ut", bufs=4))
        xp = ctx.enter_context(tc.tile_pool(name="xt", bufs=1))
        pp = ctx.enter_context(
            tc.tile_pool(name="ps", bufs=8, space=bass.MemorySpace.PSUM))
        sp = ctx.enter_context(tc.tile_pool(name="usb", bufs=1))

        for _ in range(repeat):
            # x in two SWDGE DMAs (the long-proven pattern; a 4-way split
            # hit an intermittent NRT_EXEC_UNIT_UNRECOVERABLE on HW once)
            xt8 = xp.tile([128, I_PER_CORE, NC8, B], E3M4, tag="x8")
            nc.gpsimd.dma_start(xt8[:, 0:4], xh8[:, 0:4])
            nc.gpsimd.dma_start(xt8[:, 4:8], xh8[:, 4:8])
            # U: 4 pair-DMAs alternating the two HWDGE rings (8KB/partition)
            uts = []
            for p in range(4):
                ut = up.tile([128, 2, NCH, CO], E3M4)
                eng = nc.sync if p % 2 == 0 else nc.scalar
                eng.dma_start(ut[:], uh[:, 2 * p:2 * p + 2, :, :])
                uts.append(ut)

            u_sb = sp.tile([CO, I_PER_CORE, B], F16)
            for i in range(I_PER_CORE):
                ps = pp.tile([CO, B], F32)
                ut = uts[i // 2]
                for c in range(NCH):
                    nc.tensor.matmul(ps[:], ut[:, i % 2, c, :],
                                     xt8[:, i, c, :],
                                     start=(c == 0), stop=(c == NCH - 1))
                nc.vector.tensor_copy(u_sb[:, i, :], ps[:])
            nc.gpsimd.dma_start(u_out[:], u_sb[:])
    nc.compile()
    return nc


def _build_stage_b(repeat=1):
    # K=64 (biases applied on host).  Pairs of j are packed into the 128
    # partitions (even j -> partitions 0..63, odd j -> 64..127) for both V
    # and us, so every DMA runs at full per-partition rate; matmuls address
    # the two partition halves (PE quadrant contraction).
    nc = bacc.Bacc("TRN2", target_bir_lowering=False, debug=False,
                   num_devices=N_CORES)
    NPAIR = O_PER_CORE // 2
    vh = nc.dram_tensor("vh", [128, NPAIR, S_OUT], F16,
                        kind="ExternalInput").ap()
    us = nc.dram_tensor("us", [128, NPAIR, B], F16,
                        kind="ExternalInput").ap()
    z_out = nc.dram_tensor("z_out", [128, O_PER_CORE, TT, B], F16,
                           kind="ExternalOutput").ap()

    with tile.TileContext(nc) as tc, ExitStack() as ctx:
        sb = ctx.enter_context(tc.tile_pool(name="sb", bufs=1))
        pp = ctx.enter_context(
            tc.tile_pool(name="ps", bufs=4, space=bass.MemorySpace.PSUM))

        for _ in range(repeat):
            us_all = sb.tile([128, NPAIR, B], F16, tag="us")
            nc.gpsimd.dma_start(us_all[:], us[:])
            # V quads (2 pairs = 4 j, 512KB each): g0,g1 on sync; g2,g3 on
            # scalar (behind the auto act-table load, still early enough).
            # V as 8 pair-DMAs; Act gets pairs 4,5 (behind its one-time act
            # table load it is still early), SP streams the rest in order.
            vts = [None] * NPAIR
            for p in [4, 5]:
                vt = sb.tile([128, 1, S_OUT], F16, tag="vt", bufs=8,
                             name=f"vt{p}")
                nc.scalar.dma_start(vt[:], vh[:, p:p + 1, :])
                vts[p] = vt
            for p in [0, 1, 2, 3, 6, 7]:
                vt = sb.tile([128, 1, S_OUT], F16, tag="vt", bufs=8,
                             name=f"vt{p}")
                nc.sync.dma_start(vt[:], vh[:, p:p + 1, :])
                vts[p] = vt
            # pair-granular psum (2 banks/pair, 4 pairs resident = all 8
            # banks) so the PE never stalls; one copy per pair alternating
            # DVE/Act; one z store per pair spread over Pool/SP/Act.
            zeng = [nc.gpsimd, nc.gpsimd, nc.gpsimd, nc.gpsimd,
                    nc.sync, nc.sync, nc.sync, nc.scalar]
            for p in range(NPAIR):
                vt = vts[p]
                ps = pp.tile([128, 2, TT, B], F32)
                z_sb = sb.tile([128, 2, TT, B], F16, tag="z", bufs=8)
                for h in range(2):
                    for tt in range(TT):
                        nc.tensor.matmul(ps[:, h, tt, :],
                                         vt[64 * h:64 * h + 64, 0,
                                            bass.ts(tt, 128)],
                                         us_all[64 * h:64 * h + 64, p, :],
                                         start=True, stop=True)
                if p % 2 == 0:
                    nc.vector.tensor_copy(z_sb[:], ps[:])
                else:
                    nc.scalar.copy(z_sb[:], ps[:])
                zeng[p].dma_start(z_out[:, 2 * p:2 * p + 2, :, :], z_sb[:])
    nc.compile()
    return nc


def _get(name):
    if name not in _cache:
        _cache[name] = _build_stage_a() if name == "a" else _build_stage_b()
    return _cache[name]


def _run(nc, in_maps, attempts=3):
    last = None
    for k in range(attempts):
        try:
            return run_bass_kernel_spmd(nc, in_maps, CORE_IDS).results
        except Exception as e:     # transient axon/runtime hiccups
            last = e
            if k + 1 < attempts:
                import time as _t
                _t.sleep(15 * (k + 1))
    raise last


def kernel(x, U, bU, V, bV):
    x = np.asarray(x, np.float32)
    U = np.asarray(U, np.float32)
    bU = np.asarray(bU, np.float32)
    V = np.asarray(V, np.float32)
    bV = np.asarray(bV, np.float32)

    # ---- host prep: partition-major layouts + scaled e3m4 quantization ----
    # Tuned (non-pow2) U scale: max|U| lands at 7.9 (inside the [4,8)
    # binade's fine end) — ~9% lower rms than a pow2 scale.  The scale
    # rides through psum into u (bf16), and folds out exactly via V/sU on
    # the host; it never needs to be inverted on device.
    sU = 7.85 / max(float(np.abs(U).max()), 1e-6)

    def e3(a, s):
        return np.clip(a * s, -15.5, 15.5).astype(NP_E3M4)

    # Xq: [s128, i, chunk, b], Uq: [s128, i, chunk, o]
    Xq = x.reshape(B, CI, NCH, 128).transpose(3, 1, 2, 0)
    Uq4 = U.reshape(CO, CI, NCH, 128).transpose(3, 1, 2, 0)
    # chunks < NC8 pair (U*sU/SX, x*SX); rest (U*sU, x*1) -> psum = sU*u
    Uq = e3(Uq4, sU / SX)
    X8 = e3(Xq, SX)

    in_maps_a = []
    for c in range(N_CORES):
        sl = slice(c * I_PER_CORE, (c + 1) * I_PER_CORE)
        in_maps_a.append({
            "uh": np.ascontiguousarray(Uq[:, sl]),
            "xh8": np.ascontiguousarray(X8[:, sl]),
        })

    nc_a = _get("a")
    res_a = _run(nc_a, in_maps_a)
    # u_all[o, k, b]: bf16(sU * u) straight from the device — fed to stage B
    # unmodified (no re-rounding); the sU factor is divided out of V below.
    u_all = np.concatenate(
        [res_a[c]["u_out"] for c in range(N_CORES)], axis=1)

    # ---- host mid: pair-packed V / us (even j -> partitions 0..63,
    # odd j -> 64..127); biases applied on host after stage B ----
    V3 = V.reshape(S_OUT, CO, CI)
    Vko = (V3 * (64.0 / sU)).transpose(1, 2, 0).astype(NP_F16)  # [o, k, t]
    Uko = u_all.transpose(1, 0, 2)                      # [k, o, b] bf16

    in_maps_b = []
    for c in range(N_CORES):
        sl = slice(c * O_PER_CORE, (c + 1) * O_PER_CORE)
        Vc = Vko[sl]                                    # [16, 64, 1024]
        vhc = np.concatenate([Vc[0::2], Vc[1::2]],
                             axis=1).transpose(1, 0, 2)  # [128, 8, 1024]
        uc = Uko[:, sl, :]                              # [64, 16, 64]
        usc = np.concatenate([uc[:, 0::2, :], uc[:, 1::2, :]],
                             axis=0)                    # [128, 8, 64]
        in_maps_b.append({
            "vh": np.ascontiguousarray(vhc),
            "us": np.ascontiguousarray(usc),
        })

    nc_b = _get("b")
    res_b = _run(nc_b, in_maps_b)
    # z_core[t128, j, tt, b] -> z[b, o, t] with o = c*16+j, t = tt*128 + t128
    z = np.concatenate(
        [res_b[c]["z_out"].astype(np.float32).transpose(3, 1, 2, 0)
         for c in range(N_CORES)], axis=1) * (1.0 / 64.0)  # [b, o, tt, t128]
    z = z.reshape(B, CO, S_OUT)
    # bias[o, t] = bV[t] + bU[o] * sum_i V3[t, o, i]  (exact fp32, on host)
    bias = bV[None, :] + bU[:, None] * V3.sum(-1).T
    z = z + bias[None, :, :]
    return np.ascontiguousarray(z.reshape(B, CO, 32, 32))
